# revision 1
# baseline (speedup 1.0000x reference)
"""VQ codebook kernel for Trainium2 (8 NeuronCores, SPMD over tokens).

Per core: 8192 tokens x 4096 codes x 256 dims.
  scores[n,k] = 2*z.e_k - ||e_k||^2   (argmax == argmin of L2 distance)
Matmul in 3-term bf16 split (Ah*Bh + Ah*Bl + Al*Bh, B = 2e^T) which
reproduces the fp32 argmin exactly; bias+eviction fused on DVE via
scalar_tensor_tensor; argmax via max/max_index; gather via indirect DMA.
"""
import numpy as np
import ml_dtypes
from contextlib import ExitStack

import concourse.bass as bass
import concourse.mybir as mybir
import concourse.tile as tile
import concourse.bacc as bacc
from concourse.bass_utils import run_bass_kernel_spmd

P = 128
N_TOKENS = 65536
D = 256
K = 4096
NCORES = 8
NLOC = N_TOKENS // NCORES      # 8192 tokens per core
NRG = NLOC // P                # 64 rowgroups
KT = 512                       # psum bank width
NKT = K // KT                  # 8 k-tiles

bf16 = ml_dtypes.bfloat16

_CACHE = {}


def _build():
    nc = bacc.Bacc("TRN2", target_bir_lowering=False, debug=False, num_devices=NCORES)
    ah = nc.dram_tensor("ah", [2, P, NLOC], mybir.dt.bfloat16, kind="ExternalInput").ap()
    al = nc.dram_tensor("al", [2, P, NLOC], mybir.dt.bfloat16, kind="ExternalInput").ap()
    bh = nc.dram_tensor("bh", [2, P, K], mybir.dt.bfloat16, kind="ExternalInput").ap()
    bl = nc.dram_tensor("bl", [2, P, K], mybir.dt.bfloat16, kind="ExternalInput").ap()
    esq = nc.dram_tensor("esq", [P, K], mybir.dt.float32, kind="ExternalInput").ap()
    emb = nc.dram_tensor("emb", [K, D], mybir.dt.float32, kind="ExternalInput").ap()
    zq = nc.dram_tensor("zq", [NLOC, D], mybir.dt.float32, kind="ExternalOutput").ap()
    idx_out = nc.dram_tensor("idx", [P, NRG, 8], mybir.dt.uint32, kind="ExternalOutput").ap()

    with tile.TileContext(nc) as tc, ExitStack() as ctx:
        const_pool = ctx.enter_context(tc.tile_pool(name="const", bufs=1))
        psum_pool = ctx.enter_context(tc.tile_pool(name="psum", bufs=NKT, space="PSUM"))
        score_pool = ctx.enter_context(tc.tile_pool(name="scores", bufs=2))
        small_pool = ctx.enter_context(tc.tile_pool(name="small", bufs=4))
        zq_pool = ctx.enter_context(tc.tile_pool(name="zq", bufs=4))

        ah_t = const_pool.tile([P, 2, NLOC], mybir.dt.bfloat16)
        al_t = const_pool.tile([P, 2, NLOC], mybir.dt.bfloat16)
        bh_t = const_pool.tile([P, 2, K], mybir.dt.bfloat16)
        bl_t = const_pool.tile([P, 2, K], mybir.dt.bfloat16)
        esq_t = const_pool.tile([P, K], mybir.dt.float32)
        idx_t = const_pool.tile([P, NRG, 8], mybir.dt.uint32)
        nc.sync.dma_start(bh_t[:], bh.rearrange("c p k -> p c k"))
        nc.sync.dma_start(bl_t[:], bl.rearrange("c p k -> p c k"))
        nc.sync.dma_start(esq_t[:], esq)
        # split the big A DMAs so the first rowgroups' slices land early
        ACH = NLOC // 4
        for j in range(4):
            sl = slice(j * ACH, (j + 1) * ACH)
            nc.sync.dma_start(ah_t[:, :, sl], ah.rearrange("c p t -> p c t")[:, :, sl])
            nc.sync.dma_start(al_t[:, :, sl], al.rearrange("c p t -> p c t")[:, :, sl])

        for rg in range(NRG):
            tok = slice(rg * P, (rg + 1) * P)
            psums = [
                psum_pool.tile([P, KT], mybir.dt.float32, name="ps", tag="ps")
                for _ in range(NKT)
            ]
            terms = [
                (ah_t[:, 0, tok], bh_t[:, 0, :]),
                (ah_t[:, 1, tok], bh_t[:, 1, :]),
                (ah_t[:, 0, tok], bl_t[:, 0, :]),
                (ah_t[:, 1, tok], bl_t[:, 1, :]),
                (al_t[:, 0, tok], bh_t[:, 0, :]),
                (al_t[:, 1, tok], bh_t[:, 1, :]),
            ]
            for ti, (lhsT, rhs) in enumerate(terms):
                for kt in range(NKT):
                    nc.tensor.matmul(
                        psums[kt][:],
                        lhsT=lhsT,
                        rhs=rhs[:, kt * KT:(kt + 1) * KT],
                        start=(ti == 0),
                        stop=(ti == len(terms) - 1),
                    )
            scores = score_pool.tile([P, K], mybir.dt.float32, name="scores", tag="scores")
            for kt in range(NKT):
                nc.vector.scalar_tensor_tensor(
                    out=scores[:, kt * KT:(kt + 1) * KT],
                    in0=psums[kt][:],
                    scalar=0.0,
                    in1=esq_t[:, kt * KT:(kt + 1) * KT],
                    op0=mybir.AluOpType.add,
                    op1=mybir.AluOpType.subtract,
                )
            max8 = small_pool.tile([P, 8], mybir.dt.float32, name="max8", tag="max8")
            nc.vector.max(out=max8[:], in_=scores[:])
            nc.vector.max_index(idx_t[:, rg, :], max8[:], scores[:])

            zq_t = zq_pool.tile([P, D], mybir.dt.float32, name="zqt", tag="zqt")
            nc.gpsimd.indirect_dma_start(
                out=zq_t[:],
                out_offset=None,
                in_=emb[:],
                in_offset=bass.IndirectOffsetOnAxis(ap=idx_t[:, rg, 0:1], axis=0),
            )
            nc.sync.dma_start(zq[tok, :], zq_t[:])

        nc.sync.dma_start(idx_out[:], idx_t[:])

    nc.compile()
    return nc


def _prep_shared(embedding):
    e = np.ascontiguousarray(embedding, dtype=np.float32)
    B = np.ascontiguousarray((2.0 * e).T)                     # [D, K]
    Bh = B.astype(bf16)
    Bl = (B - Bh.astype(np.float32)).astype(bf16)
    esq = np.sum(e * e, axis=1).astype(np.float32)            # [K]
    return {
        "bh": np.ascontiguousarray(Bh.reshape(2, P, K)),
        "bl": np.ascontiguousarray(Bl.reshape(2, P, K)),
        "esq": np.ascontiguousarray(np.broadcast_to(esq, (P, K))),
        "emb": e,
    }


def kernel(z, embedding):
    z = np.ascontiguousarray(z, dtype=np.float32)
    if "nc" not in _CACHE:
        _CACHE["nc"] = _build()
    nc = _CACHE["nc"]

    shared = _prep_shared(embedding)
    in_maps = []
    for c in range(NCORES):
        zc = z[c * NLOC:(c + 1) * NLOC]                       # [NLOC, D]
        A = np.ascontiguousarray(zc.T)                        # [D, NLOC]
        Ah = A.astype(bf16)
        Al = (A - Ah.astype(np.float32)).astype(bf16)
        m = dict(shared)
        m["ah"] = np.ascontiguousarray(Ah.reshape(2, P, NLOC))
        m["al"] = np.ascontiguousarray(Al.reshape(2, P, NLOC))
        in_maps.append(m)

    res = run_bass_kernel_spmd(nc, in_maps, core_ids=list(range(NCORES)))
    _CACHE["last_results"] = res

    zq = np.concatenate([res.results[c]["zq"] for c in range(NCORES)], axis=0)
    idx_parts = []
    for c in range(NCORES):
        ia = res.results[c]["idx"][:, :, 0]                   # [P, NRG]
        idx_parts.append(ia.T.reshape(-1).astype(np.int32))   # token order
    indices = np.concatenate(idx_parts, axis=0)
    return zq, indices


# revision 7
# speedup vs baseline: 3344.4925x; 3344.4925x over previous
"""VQ codebook kernel for Trainium2 (8 NeuronCores, SPMD over tokens).

Per core: 8192 tokens x 4096 codes x 256 dims.
  scores[n,k] = 2*z.e_k - ||e_k||^2   (argmax == argmin of L2 distance)
Matmul in 3-term bf16 split (Ah*Bh + Ah*Bl + Al*Bh, B = 2e^T) which
reproduces the fp32 argmin exactly; bias+eviction fused on DVE via
scalar_tensor_tensor; argmax via max/max_index; gather via indirect DMA.
"""
import numpy as np
import ml_dtypes
from contextlib import ExitStack

import jax
from jax.sharding import Mesh, PartitionSpec
from jax.experimental.shard_map import shard_map

import concourse.bass as bass
import concourse.mybir as mybir
import concourse.tile as tile
import concourse.bacc as bacc
from concourse import bass2jax

P = 128
N_TOKENS = 65536
D = 256
K = 4096
NCORES = 8
NLOC = N_TOKENS // NCORES      # 8192 tokens per core
NRG = NLOC // P                # 64 rowgroups
KT = 512                       # psum bank width
NKT = K // KT                  # 8 k-tiles

bf16 = ml_dtypes.bfloat16

_CACHE = {}


def _build(repeat=1):
    nc = bacc.Bacc("TRN2", target_bir_lowering=False, debug=False, num_devices=NCORES)
    ah = nc.dram_tensor("ah", [2, P, NLOC], mybir.dt.bfloat16, kind="ExternalInput").ap()
    al = nc.dram_tensor("al", [2, P, NLOC], mybir.dt.bfloat16, kind="ExternalInput").ap()
    bh = nc.dram_tensor("bh", [2, P, K], mybir.dt.bfloat16, kind="ExternalInput").ap()
    bl = nc.dram_tensor("bl", [2, P, K], mybir.dt.bfloat16, kind="ExternalInput").ap()
    esq = nc.dram_tensor("esq", [P, K], mybir.dt.float32, kind="ExternalInput").ap()
    emb = nc.dram_tensor("emb", [K, D], mybir.dt.float32, kind="ExternalInput").ap()
    zq = nc.dram_tensor("zq", [NLOC, D], mybir.dt.float32, kind="ExternalOutput").ap()
    idx_out = nc.dram_tensor("idx", [P, NRG, 8], mybir.dt.uint32, kind="ExternalOutput").ap()

    with tile.TileContext(nc) as tc, ExitStack() as ctx:
        const_pool = ctx.enter_context(tc.tile_pool(name="const", bufs=1))
        psum_pool = ctx.enter_context(tc.tile_pool(name="psum", bufs=NKT, space="PSUM"))
        score_pool = ctx.enter_context(tc.tile_pool(name="scores", bufs=2))
        small_pool = ctx.enter_context(tc.tile_pool(name="small", bufs=4))
        zq_pool = ctx.enter_context(tc.tile_pool(name="zq", bufs=4))

        ah_t = const_pool.tile([P, 2, NLOC], mybir.dt.bfloat16)
        al_t = const_pool.tile([P, 2, NLOC], mybir.dt.bfloat16)
        bh_t = const_pool.tile([P, 2, K], mybir.dt.bfloat16)
        bl_t = const_pool.tile([P, 2, K], mybir.dt.bfloat16)
        esq_t = const_pool.tile([P, K], mybir.dt.float32)
        idx_t = const_pool.tile([P, NRG, 8], mybir.dt.uint32)
        nc.sync.dma_start(bh_t[:], bh.rearrange("c p k -> p c k"))
        nc.sync.dma_start(bl_t[:], bl.rearrange("c p k -> p c k"))
        nc.sync.dma_start(esq_t[:], esq)
        # split the big A DMAs so the first rowgroups' slices land early
        ACH = NLOC // 4
        for j in range(4):
            sl = slice(j * ACH, (j + 1) * ACH)
            nc.sync.dma_start(ah_t[:, :, sl], ah.rearrange("c p t -> p c t")[:, :, sl])
            nc.sync.dma_start(al_t[:, :, sl], al.rearrange("c p t -> p c t")[:, :, sl])

        def body():
            _emit_passes(nc, tc, ah_t, al_t, bh_t, bl_t, esq_t, idx_t, emb, zq,
                         psum_pool, score_pool, small_pool, zq_pool)

        if repeat == 1:
            body()
        else:
            with tc.For_i(0, repeat, 1):
                body()

        nc.sync.dma_start(idx_out[:], idx_t[:])

    nc.compile()
    return nc


def _emit_passes(nc, tc, ah_t, al_t, bh_t, bl_t, esq_t, idx_t, emb, zq,
                 psum_pool, score_pool, small_pool, zq_pool):
        for rg in range(NRG):
            tok = slice(rg * P, (rg + 1) * P)
            psums = [
                psum_pool.tile([P, KT], mybir.dt.float32, name="ps", tag="ps")
                for _ in range(NKT)
            ]
            terms = [
                (ah_t[:, 0, tok], bh_t[:, 0, :]),
                (ah_t[:, 1, tok], bh_t[:, 1, :]),
                (ah_t[:, 0, tok], bl_t[:, 0, :]),
                (ah_t[:, 1, tok], bl_t[:, 1, :]),
                (al_t[:, 0, tok], bh_t[:, 0, :]),
                (al_t[:, 1, tok], bh_t[:, 1, :]),
            ]
            for ti, (lhsT, rhs) in enumerate(terms):
                for kt in range(NKT):
                    nc.tensor.matmul(
                        psums[kt][:],
                        lhsT=lhsT,
                        rhs=rhs[:, kt * KT:(kt + 1) * KT],
                        start=(ti == 0),
                        stop=(ti == len(terms) - 1),
                    )
            scores = score_pool.tile([P, K], mybir.dt.float32, name="scores", tag="scores")
            for kt in range(NKT):
                nc.vector.scalar_tensor_tensor(
                    out=scores[:, kt * KT:(kt + 1) * KT],
                    in0=psums[kt][:],
                    scalar=0.0,
                    in1=esq_t[:, kt * KT:(kt + 1) * KT],
                    op0=mybir.AluOpType.add,
                    op1=mybir.AluOpType.subtract,
                )
            max8 = small_pool.tile([P, 8], mybir.dt.float32, name="max8", tag="max8")
            nc.vector.max(out=max8[:], in_=scores[:])
            nc.vector.max_index(idx_t[:, rg, :], max8[:], scores[:])

            zq_t = zq_pool.tile([P, D], mybir.dt.float32, name="zqt", tag="zqt")
            nc.gpsimd.indirect_dma_start(
                out=zq_t[:],
                out_offset=None,
                in_=emb[:],
                in_offset=bass.IndirectOffsetOnAxis(ap=idx_t[:, rg, 0:1], axis=0),
            )
            nc.sync.dma_start(zq[tok, :], zq_t[:])


def _make_runner(nc):
    """Cached multi-core PJRT runner (run_bass_via_pjrt with the jit hoisted)."""
    bass2jax.install_neuronx_cc_hook()
    partition_name = nc.partition_id_tensor.name if nc.partition_id_tensor else None

    in_names, out_names, out_avals, zero_shapes = [], [], [], []
    for alloc in nc.m.functions[0].allocations:
        if not isinstance(alloc, mybir.MemoryLocationSet):
            continue
        name = alloc.memorylocations[0].name
        if alloc.kind == "ExternalInput":
            if name != partition_name:
                in_names.append(name)
        elif alloc.kind == "ExternalOutput":
            out_names.append(name)
            shape = tuple(alloc.tensor_shape)
            dtype = mybir.dt.np(alloc.dtype)
            out_avals.append(jax.core.ShapedArray(shape, dtype))
            zero_shapes.append((shape, dtype))
    n_params = len(in_names)
    n_outs = len(out_avals)
    all_in_names = list(in_names) + list(out_names)
    if partition_name is not None:
        all_in_names.append(partition_name)
    donate = tuple(range(n_params, n_params + n_outs))

    def _body(*args):
        operands = list(args)
        if partition_name is not None:
            operands.append(bass2jax.partition_id_tensor())
        outs = bass2jax._bass_exec_p.bind(
            *operands,
            out_avals=tuple(out_avals),
            in_names=tuple(all_in_names),
            out_names=tuple(out_names),
            lowering_input_output_aliases=(),
            sim_require_finite=True,
            sim_require_nnan=True,
            nc=nc,
        )
        return tuple(outs)

    devices = jax.devices()[:NCORES]
    mesh = Mesh(np.asarray(devices), ("core",))
    in_specs = (PartitionSpec("core"),) * (n_params + n_outs)
    out_specs = (PartitionSpec("core"),) * n_outs
    sharded = jax.jit(
        shard_map(_body, mesh=mesh, in_specs=in_specs, out_specs=out_specs,
                  check_rep=False),
        donate_argnums=donate,
        keep_unused=True,
    )

    def run(in_maps):
        concat_in = [
            np.concatenate([np.asarray(in_maps[c][nm]) for c in range(NCORES)], axis=0)
            for nm in in_names
        ]
        concat_zeros = [
            np.zeros((NCORES * s[0], *s[1:]), dt) for (s, dt) in zero_shapes
        ]
        out_arrs = sharded(*concat_in, *concat_zeros)
        out_arrs = [np.asarray(a) for a in out_arrs]
        return [
            {nm: out_arrs[i].reshape(NCORES, *out_avals[i].shape)[c]
             for i, nm in enumerate(out_names)}
            for c in range(NCORES)
        ]

    return run


def _prep_shared(embedding):
    e = np.ascontiguousarray(embedding, dtype=np.float32)
    B = np.ascontiguousarray((2.0 * e).T)                     # [D, K]
    Bh = B.astype(bf16)
    Bl = (B - Bh.astype(np.float32)).astype(bf16)
    esq = np.sum(e * e, axis=1).astype(np.float32)            # [K]
    return {
        "bh": np.ascontiguousarray(Bh.reshape(2, P, K)),
        "bl": np.ascontiguousarray(Bl.reshape(2, P, K)),
        "esq": np.ascontiguousarray(np.broadcast_to(esq, (P, K))),
        "emb": e,
    }


def _get_runner(repeat=1):
    key = ("run", repeat)
    if key not in _CACHE:
        _CACHE[key] = _make_runner(_build(repeat=repeat))
    return _CACHE[key]


def _in_maps(z, embedding):
    shared = _prep_shared(embedding)
    in_maps = []
    for c in range(NCORES):
        zc = z[c * NLOC:(c + 1) * NLOC]                       # [NLOC, D]
        A = np.ascontiguousarray(zc.T)                        # [D, NLOC]
        Ah = A.astype(bf16)
        Al = (A - Ah.astype(np.float32)).astype(bf16)
        m = dict(shared)
        m["ah"] = np.ascontiguousarray(Ah.reshape(2, P, NLOC))
        m["al"] = np.ascontiguousarray(Al.reshape(2, P, NLOC))
        in_maps.append(m)
    return in_maps


def kernel(z, embedding, repeat=1):
    z = np.ascontiguousarray(z, dtype=np.float32)
    run = _get_runner(repeat)
    results = run(_in_maps(z, embedding))

    zq = np.concatenate([results[c]["zq"] for c in range(NCORES)], axis=0)
    idx_parts = []
    for c in range(NCORES):
        ia = results[c]["idx"][:, :, 0]                       # [P, NRG]
        idx_parts.append(ia.T.reshape(-1).astype(np.int32))   # token order
    indices = np.concatenate(idx_parts, axis=0)
    return zq, indices
